# revision 37
# baseline (speedup 1.0000x reference)
"""AdaptiveSparseSelfAttention TRN2 kernel (8 NeuronCores, SPMD).

Sharding: core c handles batch b = c//2 and T-half th = c%2 (1024 query rows).
Host rotates x per core so that rows 0:1024 of the per-core "x" input are that
core's query rows; k/v use all 2048 rows (attention is order-invariant over s).

Pipeline (fp32/f32r projection, fp16 attention path):
  stage 1/2: x -> xT (PE transpose) -> q,k,v = Wqkv^T @ xT as float32r
      matmuls (1 cyc/row vs 4 for fp32); q (pre-scaled 1/8), k cast to fp16,
      v packed per head as [128s, 16sc, 64d | ones] fp16 (ones column yields
      softmax denominators for free during PV).
  per head: S = q.k computed twice from the SAME fp16 q/k tiles
      (q-stationary [t,s] for selection, k-stationary [s,t] for PV) so both
      fp32 psum layouts are bitwise identical; e = fp16(exp(S)) likewise.
    selection: e_S -> top-8 per 22 segments (DVE max8, no replace) -> 176
      candidates -> exact 64th-largest via 7x(max8+match_replace)+max8.
      Candidates hold the exact top-64 unless a segment has >8 of them
      (rare; over-includes only, never drops).
    tau row -> PE transpose -> ETh[s,t] broadcast (K=1 matmul).
    streaming S^T (head-pair row-packed K=64 matmuls) -> e (ACT exp) ->
      dense PV; esp = (e >= ETh) * e (2 fp16 DVE ops) -> sparse PV.
    combine: oh = g*den/Zd + (1-g)*sp/Zsp: g folded via per-partition ACT
      scale, Z broadcast to partitions 0:64 by K=1 matmul, then
      reciprocal_approx_fast at base partition 0 (broken off base 0).
  out = sum_h oh_h^T @ Wout[h] (fp16) -> DMA out.

Software pipelining: the whole kernel is emitted as 8-slot rounds per head
pair; slot = [C/D chunks of pair p] + [S matmuls+exps of pair p+1] +
[tournament of pair p+1], so the DVE tournament (the bottleneck, ~55% busy
share) overlaps PE/ACT of the previous pair's PV phase. Stage-2 chunks are
spread across pair-0's slots; ETh is emitted per head as soon as its last
tournament tile finishes; the first half of the output projection is
interleaved into the last pair's slots.
"""

import os
import numpy as np

DIM, NHEAD, TOPK, DK = 512, 8, 64, 64
B, T = 4, 2048
H = NHEAD
TQ = T // 2  # query rows per core
NCORES = 8

_CACHE = {}
LAST_EXEC_NS = None


def _build_nc():
    from contextlib import ExitStack
    import concourse.bass as bass
    import concourse.tile as tile
    from concourse import bacc, mybir
    from concourse.masks import make_identity

    f32 = mybir.dt.float32
    f16 = mybir.dt.float16
    f32r = mybir.dt.float32r
    AF = mybir.ActivationFunctionType
    OP = mybir.AluOpType

    # 22 round-1 segments over 2048 (2x94 + 20x93)
    SEGB = []
    _off = 0
    for _i in range(22):
        _w = 94 if _i < 2 else 93
        SEGB.append((_off, _off + _w))
        _off += _w
    assert _off == T

    nc = bacc.Bacc("TRN2", target_bir_lowering=False, debug=False)
    x_ext = nc.declare_dram_parameter("x", [T, DIM], f32, isOutput=False)
    wqkv_ext = nc.declare_dram_parameter("wqkv", [DIM, 3 * DIM], f32, isOutput=False)
    wout_ext = nc.declare_dram_parameter("wout", [DIM, DIM], f32, isOutput=False)
    alpha_ext = nc.declare_dram_parameter("alpha", [1, H], f32, isOutput=False)
    out_ext = nc.declare_dram_parameter("out", [TQ, DIM], f32, isOutput=True)

    with tile.TileContext(nc) as tc, ExitStack() as ctx:
        consts = ctx.enter_context(tc.tile_pool(name="consts", bufs=1))
        wpool = ctx.enter_context(tc.tile_pool(name="weights", bufs=1))
        qkp = ctx.enter_context(tc.tile_pool(name="qk", bufs=1))
        vzp = ctx.enter_context(tc.tile_pool(name="v", bufs=1))
        # PSUM pools: 3 + 4 + 1 = 8 banks
        ppa = ctx.enter_context(tc.tile_pool(name="ppa", bufs=3, space="PSUM"))
        ppv = ctx.enter_context(tc.tile_pool(name="ppv", bufs=1, space="PSUM"))
        ppb = ctx.enter_context(tc.tile_pool(name="ppb", bufs=1, space="PSUM"))
        # per-head pools (stage pools stay open until interleaved stage-2 done)
        epool = ctx.enter_context(tc.tile_pool(name="eS", bufs=2))
        candp = ctx.enter_context(tc.tile_pool(name="cand", bufs=2))
        v8p = ctx.enter_context(tc.tile_pool(name="v8", bufs=2))
        v8fp = ctx.enter_context(tc.tile_pool(name="v8f", bufs=18))
        thp = ctx.enter_context(tc.tile_pool(name="theta", bufs=1))
        ethp = ctx.enter_context(tc.tile_pool(name="eth", bufs=2))
        ecp = ctx.enter_context(tc.tile_pool(name="ec", bufs=2))
        esp_p = ctx.enter_context(tc.tile_pool(name="esp", bufs=2))
        dpool = ctx.enter_context(tc.tile_pool(name="comb", bufs=1))
        zpool = ctx.enter_context(tc.tile_pool(name="z", bufs=1))
        ohpool = ctx.enter_context(tc.tile_pool(name="oh", bufs=1))
        opool = ctx.enter_context(tc.tile_pool(name="out", bufs=1))
        sctx = ExitStack()
        wqp = sctx.enter_context(tc.tile_pool(name="wqkv", bufs=1))
        xload = sctx.enter_context(tc.tile_pool(name="xload", bufs=3))
        xtp = sctx.enter_context(tc.tile_pool(name="xT", bufs=1))
        wst = sctx.enter_context(tc.tile_pool(name="wst", bufs=1))

        # ---- constants ----
        ident = consts.tile([128, 128], f32)
        make_identity(nc, ident)
        ones16 = consts.tile([1, 128], f16)
        nc.vector.memset(ones16, 1.0)

        alpha_sb = consts.tile([1, H], f32)
        nc.sync.dma_start(out=alpha_sb, in_=alpha_ext[:])
        g_sb = consts.tile([1, H], f32)
        nc.scalar.activation(g_sb, alpha_sb, AF.Sigmoid)
        gm1_sb = consts.tile([1, H], f32)  # 1 - g
        nc.vector.tensor_scalar(gm1_sb, g_sb, -1.0, 1.0, OP.mult, op1=OP.add)
        ones65 = consts.tile([65, 64], f16)  # row 64 used as K=1 bcast lhsT
        nc.vector.memset(ones65, 1.0)
        onesf = consts.tile([1, 64], f32)
        nc.vector.memset(onesf, 1.0)
        # gcol[:, h] = g_h on all 64 partitions (ACT scale operand); [:, H+h] = 1-g_h
        ps_g = ppb.tile([128, 512], f32, tag="bc")
        nc.tensor.matmul(ps_g[0:64, 0:H], onesf, g_sb)
        nc.tensor.matmul(ps_g[0:64, H:2 * H], onesf, gm1_sb)
        gcol = consts.tile([64, 2 * H], f32)
        nc.scalar.activation(gcol, ps_g[0:64, 0:2 * H], AF.Copy)

        # ---- weights ----
        wqkv_sb = []
        for kc in range(4):
            st_ = wst.tile([128, 3 * DIM], f32, tag="wqst", name="wqst")
            nc.sync.dma_start(out=st_, in_=wqkv_ext[kc * 128:(kc + 1) * 128, :])
            t_ = wqp.tile([128, 3 * DIM], f32r, tag=f"wqkv{kc}", name=f"wqkv{kc}")
            nc.scalar.activation(t_, st_, AF.Copy)
            wqkv_sb.append(t_)
        woutP = []

        def emit_wout():
            for hh in range(H):
                st_ = wst.tile([64, DIM], f32, tag="wstage")
                nc.sync.dma_start(out=st_, in_=wout_ext[hh * 64:(hh + 1) * 64, :])
                t_ = wpool.tile([64, DIM], f16, tag=f"wout{hh}", name=f"wout{hh}")
                nc.scalar.activation(t_, st_, AF.Copy)
                woutP.append(t_)

        # ---- stage 1: x -> xT [512, 2048] fp32 ----
        xTbig = xtp.tile([128, 4, T], f32r, tag="xT", name="xT")
        def emit_stage1(i):
            xt = xload.tile([128, DIM], f32, tag="xt")
            nc.sync.dma_start(out=xt, in_=x_ext[i * 128:(i + 1) * 128, :])
            ps = ppa.tile([128, 4, 128], f32, tag="mm", name="mm")
            for j in range(4):
                nc.tensor.transpose(ps[:, j, :], xt[:, j * 128:(j + 1) * 128], ident)
            nc.scalar.activation(xTbig[:, :, i * 128:(i + 1) * 128], ps, AF.Copy)

        # ---- stage 2 (fp32 matmuls): qT (scaled 1/8), kT, v -> fp16 ----
        # emitted per-m so pair-0 phase A can interleave with m=1..3 and v
        q16 = [qkp.tile([128, TQ], f16, tag=f"q{m}", name=f"q{m}") for m in range(4)]
        k16 = [qkp.tile([128, T], f16, tag=f"k{m}", name=f"k{m}") for m in range(4)]

        def emit_qk_chunk(m, j):
            # j 0..1: q chunks; j 2..5: k chunks
            if j < 2:
                nb = j
                ps = ppa.tile([128, 512], f32, tag="mm")
                for kc in range(4):
                    nc.tensor.matmul(ps, wqkv_sb[kc][:, m * 128:(m + 1) * 128],
                                     xTbig[:, kc, nb * 512:(nb + 1) * 512],
                                     start=(kc == 0), stop=(kc == 3))
                nc.scalar.activation(q16[m][:, nb * 512:(nb + 1) * 512], ps,
                                     AF.Copy, scale=0.125)
            else:
                nb = j - 2
                ps = ppa.tile([128, 512], f32, tag="mm")
                for kc in range(4):
                    nc.tensor.matmul(ps, wqkv_sb[kc][:, DIM + m * 128:DIM + (m + 1) * 128],
                                     xTbig[:, kc, nb * 512:(nb + 1) * 512],
                                     start=(kc == 0), stop=(kc == 3))
                nc.scalar.activation(k16[m][:, nb * 512:(nb + 1) * 512], ps, AF.Copy)

        def emit_qk(m):
            for j in range(6):
                emit_qk_chunk(m, j)

        v65 = []
        for hh in range(H):
            t_ = vzp.tile([128, 16, 65], f16, tag=f"v65{hh}", name=f"v65{hh}")
            nc.vector.memset(t_[:, :, 64:65], 1.0)
            v65.append(t_)

        def emit_v(st):
            ps = ppa.tile([128, 512], f32, tag="mm")
            for kc in range(4):
                nc.tensor.matmul(ps, xTbig[:, kc, st * 128:(st + 1) * 128],
                                 wqkv_sb[kc][:, 2 * DIM:3 * DIM],
                                 start=(kc == 0), stop=(kc == 3))
            for hh in range(H):
                nc.scalar.activation(v65[hh][:, st, 0:64],
                                     ps[:, hh * 64:(hh + 1) * 64], AF.Copy)

        oh16 = [ohpool.tile([64, TQ], f16, tag=f"oh{hh}", name=f"oh{hh}")
                for hh in range(H)]

        # ---- software-pipelined main loop ----
        # emit_A(p, tt): S matmuls + exp + tournament for both heads of a
        #   128-query-row tile. Round 1 = top-8 per 64-wide segment (32 max8,
        #   no replace): candidates hold the exact top-64 unless a segment
        #   contains >8 of them (~5e-3 of rows, over-include only).
        # emit_B(p): tau rows -> ETh broadcast.
        # emit_C(p, tb, sc): S^T chunk -> e -> dense PV; mask -> sparse PV.
        # emit_D(p, tb): combine into oh16.
        # Schedule: A(0); B(0); then for each p: interleave A(p+1) tiles with
        # C(p) chunks so the DVE tournament of p+1 overlaps PE/ACT of C(p).
        thc = {}
        ETh = {}

        eS_live = {}

        def emit_A_mm(p, tt):
            pair = (2 * p, 2 * p + 1)
            eS = {}
            for hh in pair:
                eS[hh] = epool.tile([128, T], f16, tag=f"eS{hh % 2}",
                                    name=f"eS{hh % 2}")
            for nb in range(4):
                for hh in pair:
                    rr = (hh % 2) * 64
                    ps = ppa.tile([128, 512], f32, tag="mm")
                    nc.tensor.matmul(ps, q16[p][rr:rr + 64, tt * 128:(tt + 1) * 128],
                                     k16[p][rr:rr + 64, nb * 512:(nb + 1) * 512])
                    nc.scalar.activation(eS[hh][:, nb * 512:(nb + 1) * 512],
                                         ps, AF.Exp)
            eS_live[p] = eS

        def emit_A_tour(p, tt):
            pair = (2 * p, 2 * p + 1)
            eS = eS_live[p]
            for hh in pair:
                if tt == 7 and hh == pair[1]:
                    emit_B_head(pair[0])
                e_t = eS[hh]
                cand = candp.tile([128, 176], f16, tag=f"cand{hh % 2}",
                                  name=f"cand{hh % 2}")
                for gs, (lo, hi) in enumerate(SEGB):
                    nc.vector.max(out=cand[:, gs * 8:gs * 8 + 8],
                                  in_=e_t[:, lo:hi])
                for r in range(7):
                    v8 = v8p.tile([128, 8], f16, tag=f"v8{hh % 2}",
                                  name=f"v8{hh % 2}")
                    nc.vector.max(out=v8, in_=cand)
                    nc.vector.match_replace(out=cand, in_to_replace=v8,
                                            in_values=cand, imm_value=0.0)
                v8f = v8fp.tile([128, 8], f32, tag="v8f", name="v8f")
                nc.vector.max(out=v8f, in_=cand)
                thc[hh].append(v8f[:, 7:8])
            if tt == 7:
                emit_B_head(pair[1])

        def emit_B_head(hh):
            thetaR = thp.tile([1, TQ], f16, tag=f"thR{hh % 2}",
                              name=f"thR{hh % 2}")
            for half in range(2):
                psr = ppb.tile([128, 512], f32, tag="bc")
                for q4 in range(4):
                    tt = half * 4 + q4
                    nc.tensor.transpose(psr[0:1, q4 * 128:(q4 + 1) * 128],
                                        thc[hh][tt], ident)
                nc.scalar.activation(thetaR[0:1, half * 512:(half + 1) * 512],
                                     psr[0:1, :], AF.Copy)
            ETh[hh % 2] = ethp.tile([128, TQ], f16, tag=f"ETh{hh % 2}",
                                    name=f"ETh{hh % 2}")
            for nb in range(2):
                psb = ppb.tile([128, 512], f32, tag="bc")
                nc.tensor.matmul(psb, ones16,
                                 thetaR[0:1, nb * 512:(nb + 1) * 512])
                nc.scalar.activation(ETh[hh % 2][:, nb * 512:(nb + 1) * 512],
                                     psb, AF.Copy)

        def emit_C(p, tb, sc, den, sp):
            pair = (2 * p, 2 * p + 1)
            tbs = slice(tb * 512, (tb + 1) * 512)
            pss, ec = {}, {}
            for hh in pair:
                rr = (hh % 2) * 64
                ps = ppa.tile([128, 512], f32, tag="mm")
                nc.tensor.matmul(ps, k16[p][rr:rr + 64, sc * 128:(sc + 1) * 128],
                                 q16[p][rr:rr + 64, tbs])
                pss[hh] = ps
            for hh in pair:
                ec[hh] = ecp.tile([128, 512], f16, tag=f"ec{hh % 2}",
                                  name=f"ec{hh % 2}")
                nc.scalar.activation(ec[hh], pss[hh], AF.Exp)
                nc.tensor.matmul(den[hh][0:65, :], v65[hh][:, sc, :], ec[hh],
                                 start=(sc == 0), stop=(sc == 15))
            for hh in pair:
                es = esp_p.tile([128, 512], f16, tag=f"esp{hh % 2}",
                                name=f"esp{hh % 2}")
                nc.vector.tensor_tensor(es, ec[hh], ETh[hh % 2][:, tbs],
                                        op=OP.is_ge)
                nc.vector.tensor_tensor(es, es, ec[hh], op=OP.mult)
                nc.tensor.matmul(sp[hh][0:65, :], v65[hh][:, sc, :], es,
                                 start=(sc == 0), stop=(sc == 15))

        def emit_D(p, tb, den, sp):
            pair = (2 * p, 2 * p + 1)
            tbs = slice(tb * 512, (tb + 1) * 512)
            for hh in pair:
                d16 = dpool.tile([64, 512], f16, tag=f"d16{hh % 2}",
                                 name=f"d16{hh % 2}")
                nc.scalar.activation(d16, den[hh][0:64, :], AF.Copy,
                                     scale=gcol[:, hh:hh + 1])
                s16 = dpool.tile([64, 512], f16, tag=f"s16{hh % 2}",
                                 name=f"s16{hh % 2}")
                nc.scalar.activation(s16, sp[hh][0:64, :], AF.Copy,
                                     scale=gcol[:, H + hh:H + hh + 1])
                zr = zpool.tile([65, 2 * 512], f16, tag=f"zr{hh % 2}",
                                name=f"zr{hh % 2}")
                nc.scalar.activation(zr[64:65, 0:512], den[hh][64:65, :], AF.Copy)
                nc.scalar.activation(zr[64:65, 512:1024], sp[hh][64:65, :], AF.Copy)
                bcd_ps = ppb.tile([128, 512], f32, tag="bc")
                nc.tensor.matmul(bcd_ps[0:64, :], ones65[64:65, :],
                                 zr[64:65, 0:512])
                zbd = zpool.tile([64, 512], f32, tag=f"zbd{hh % 2}",
                                 name=f"zbd{hh % 2}")
                nc.vector.reciprocal_approx_fast(zbd, bcd_ps[0:64, :])
                bcd = dpool.tile([64, 512], f16, tag=f"bcd{hh % 2}",
                                 name=f"bcd{hh % 2}")
                nc.scalar.activation(bcd, zbd, AF.Copy)
                bcs_ps = ppb.tile([128, 512], f32, tag="bc")
                nc.tensor.matmul(bcs_ps[0:64, :], ones65[64:65, :],
                                 zr[64:65, 512:1024])
                zbs = zpool.tile([64, 512], f32, tag=f"zbs{hh % 2}",
                                 name=f"zbs{hh % 2}")
                nc.vector.reciprocal_approx_fast(zbs, bcs_ps[0:64, :])
                bcs = dpool.tile([64, 512], f16, tag=f"bcs{hh % 2}",
                                 name=f"bcs{hh % 2}")
                nc.scalar.activation(bcs, zbs, AF.Copy)
                tmp = dpool.tile([64, 512], f16, tag=f"tmp{hh % 2}",
                                 name=f"tmp{hh % 2}")
                nc.vector.tensor_tensor(tmp, d16, bcd, op=OP.mult)
                nc.vector.tensor_tensor(oh16[hh][:, tbs], s16, bcs, op=OP.mult)
                nc.vector.tensor_add(oh16[hh][:, tbs], oh16[hh][:, tbs], tmp)

        def emit_out(tt):
            psA = ppa.tile([128, 512], f32, tag="mm")
            for hh in range(H):
                nc.tensor.matmul(psA, oh16[hh][:, tt * 128:(tt + 1) * 128],
                                 woutP[hh], start=(hh == 0), stop=(hh == 7))
            o_sb = opool.tile([128, DIM], f32, tag="osb")
            nc.scalar.activation(o_sb, psA, AF.Copy)
            nc.sync.dma_start(out=out_ext[tt * 128:(tt + 1) * 128, :], in_=o_sb)

        thc = {0: [], 1: []}
        for blk in range(4):
            for i4 in range(4):
                emit_stage1(blk * 4 + i4)
            # q0/k0 chunks over t-range blk*512 are ready now
            if blk in (0, 2):
                emit_qk_chunk(0, blk // 2)
            emit_qk_chunk(0, 2 + blk)
        stage2_rest = ([("qk", (m, j)) for m in (1, 2, 3) for j in range(6)]
                       + [("v", st) for st in range(16)])
        si = 0
        takes = [5, 5, 4, 4, 4, 4, 4, 4]
        for tt in range(8):
            emit_A_mm(0, tt)
            for _ in range(takes[tt]):
                if si < len(stage2_rest):
                    kind, arg = stage2_rest[si]
                    si += 1
                    if kind == "qk":
                        emit_qk_chunk(*arg)
                    else:
                        emit_v(arg)
            emit_A_tour(0, tt)
        assert si == len(stage2_rest)
        emit_wout()
        sctx.close()
        for p in range(4):
            den, sp = {}, {}
            nxt = p + 1
            if nxt < 4:
                thc = {2 * nxt: [], 2 * nxt + 1: []}
            for tt in range(8):
                ci0 = tt * 4
                pend_d = []
                for ci in range(ci0, ci0 + 4):
                    tb, sc = ci // 16, ci % 16
                    if sc == 0:
                        for hh in (2 * p, 2 * p + 1):
                            den[hh] = ppv.tile([65, 512], f32, tag=f"den{hh % 2}",
                                               name=f"den{hh % 2}")
                            sp[hh] = ppv.tile([65, 512], f32, tag=f"sp{hh % 2}",
                                              name=f"sp{hh % 2}")
                    emit_C(p, tb, sc, den, sp)
                    if sc == 15:
                        # defer the combine past this slot's tournament so its
                        # ACT/PE-latency chain doesn't head-of-line block the
                        # DVE queue
                        pend_d.append((tb, dict(den), dict(sp)))
                if nxt < 4:
                    emit_A_mm(nxt, tt)
                if p == 3 and tt >= 4:
                    emit_out(tt - 4)
                if nxt < 4:
                    emit_A_tour(nxt, tt)
                for tb_, dd_, ss_ in pend_d:
                    emit_D(p, tb_, dd_, ss_)
        for tt in range(4, 8):
            emit_out(tt)


    nc.finalize()
    return nc


def kernel(x, Wqkv, Wout, alpha):
    global LAST_EXEC_NS
    from concourse.bass_utils import run_bass_kernel_spmd

    if "nc" not in _CACHE:
        _CACHE["nc"] = _build_nc()
    nc = _CACHE["nc"]

    x = np.ascontiguousarray(np.asarray(x, dtype=np.float32))
    wqkv = np.ascontiguousarray(np.asarray(Wqkv, dtype=np.float32))
    wout = np.ascontiguousarray(np.asarray(Wout, dtype=np.float32))
    al = np.ascontiguousarray(np.asarray(alpha, dtype=np.float32).reshape(1, H))

    in_maps = []
    for c in range(NCORES):
        b, th = c // 2, c % 2
        xb = x[b]
        if th == 1:
            xb = np.ascontiguousarray(np.concatenate([xb[TQ:], xb[:TQ]], axis=0))
        in_maps.append({"x": xb, "wqkv": wqkv, "wout": wout, "alpha": al})

    trace = bool(int(os.environ.get("KERNEL_PROFILE", "0")))
    if trace:
        # this container's antenv lacks axon_hooks; shim it with the ctypes
        # NTFF hook from trn_agent_boot so neuron-profile tracing works
        import sys as _sys, types as _types
        if "antenv.axon_hooks" not in _sys.modules:
            try:
                from antenv.axon_hooks import get_axon_ntff_profile_hook  # noqa
            except ImportError:
                _hook = None
                try:
                    from trn_agent_boot.trn_boot import _ntff_profile_via_ctypes
                    _hook = _ntff_profile_via_ctypes("/opt/axon/libaxon_pjrt.so")
                except Exception:
                    _hook = None
                _m = _types.ModuleType("antenv.axon_hooks")
                _m.get_axon_ntff_profile_hook = lambda: _hook
                _sys.modules["antenv.axon_hooks"] = _m
    res = run_bass_kernel_spmd(nc, in_maps, core_ids=list(range(NCORES)),
                               trace=trace)
    LAST_EXEC_NS = res.exec_time_ns
    if trace:
        _CACHE["last_results"] = res

    out = np.empty((B, T, DIM), np.float32)
    for c in range(NCORES):
        b, th = c // 2, c % 2
        out[b, th * TQ:(th + 1) * TQ, :] = res.results[c]["out"]
    return out


# revision 38
# speedup vs baseline: 1.0058x; 1.0058x over previous
"""AdaptiveSparseSelfAttention TRN2 kernel (8 NeuronCores, SPMD).

Sharding: core c handles batch b = c//2 and T-half th = c%2 (1024 query rows).
Host rotates x per core so that rows 0:1024 of the per-core "x" input are that
core's query rows; k/v use all 2048 rows (attention is order-invariant over s).

Pipeline (fp32/f32r projection, fp16 attention path):
  stage 1/2: x -> xT (PE transpose) -> q,k,v = Wqkv^T @ xT as float32r
      matmuls (1 cyc/row vs 4 for fp32); q (pre-scaled 1/8), k cast to fp16,
      v packed per head as [128s, 16sc, 64d | ones] fp16 (ones column yields
      softmax denominators for free during PV).
  per head: S = q.k computed twice from the SAME fp16 q/k tiles
      (q-stationary [t,s] for selection, k-stationary [s,t] for PV) so both
      fp32 psum layouts are bitwise identical; e = fp16(exp(S)) likewise.
    selection: e_S -> top-8 per 22 segments (DVE max8, no replace) -> 176
      candidates -> exact 64th-largest via 7x(max8+match_replace)+max8.
      Candidates hold the exact top-64 unless a segment has >8 of them
      (rare; over-includes only, never drops).
    tau row -> PE transpose -> ETh[s,t] broadcast (K=1 matmul).
    streaming S^T (head-pair row-packed K=64 matmuls) -> e (ACT exp) ->
      dense PV; esp = (e >= ETh) * e (2 fp16 DVE ops) -> sparse PV.
    combine: oh = g*den/Zd + (1-g)*sp/Zsp: g folded via per-partition ACT
      scale, Z broadcast to partitions 0:64 by K=1 matmul, then
      reciprocal_approx_fast at base partition 0 (broken off base 0).
  out = sum_h oh_h^T @ Wout[h] (fp16) -> DMA out.

Software pipelining: the whole kernel is emitted as 8-slot rounds per head
pair; slot = [C/D chunks of pair p] + [S matmuls+exps of pair p+1] +
[tournament of pair p+1], so the DVE tournament (the bottleneck, ~55% busy
share) overlaps PE/ACT of the previous pair's PV phase. Stage-2 chunks are
spread across pair-0's slots; ETh is emitted per head as soon as its last
tournament tile finishes; the first half of the output projection is
interleaved into the last pair's slots.
"""

import os
import numpy as np

DIM, NHEAD, TOPK, DK = 512, 8, 64, 64
B, T = 4, 2048
H = NHEAD
TQ = T // 2  # query rows per core
NCORES = 8

_CACHE = {}
LAST_EXEC_NS = None


def _build_nc():
    from contextlib import ExitStack
    import concourse.bass as bass
    import concourse.tile as tile
    from concourse import bacc, mybir
    from concourse.masks import make_identity

    f32 = mybir.dt.float32
    f16 = mybir.dt.float16
    f32r = mybir.dt.float32r
    AF = mybir.ActivationFunctionType
    OP = mybir.AluOpType

    # 22 round-1 segments over 2048 (2x94 + 20x93)
    SEGB = []
    _off = 0
    for _i in range(22):
        _w = 94 if _i < 2 else 93
        SEGB.append((_off, _off + _w))
        _off += _w
    assert _off == T

    nc = bacc.Bacc("TRN2", target_bir_lowering=False, debug=False)
    x_ext = nc.declare_dram_parameter("x", [T, DIM], f32, isOutput=False)
    wqkv_ext = nc.declare_dram_parameter("wqkv", [DIM, 3 * DIM], f32, isOutput=False)
    wout_ext = nc.declare_dram_parameter("wout", [DIM, DIM], f32, isOutput=False)
    alpha_ext = nc.declare_dram_parameter("alpha", [1, H], f32, isOutput=False)
    out_ext = nc.declare_dram_parameter("out", [TQ, DIM], f32, isOutput=True)

    with tile.TileContext(nc) as tc, ExitStack() as ctx:
        consts = ctx.enter_context(tc.tile_pool(name="consts", bufs=1))
        wpool = ctx.enter_context(tc.tile_pool(name="weights", bufs=1))
        qkp = ctx.enter_context(tc.tile_pool(name="qk", bufs=1))
        vzp = ctx.enter_context(tc.tile_pool(name="v", bufs=1))
        # PSUM pools: 3 + 4 + 1 = 8 banks
        ppa = ctx.enter_context(tc.tile_pool(name="ppa", bufs=3, space="PSUM"))
        ppv = ctx.enter_context(tc.tile_pool(name="ppv", bufs=1, space="PSUM"))
        ppb = ctx.enter_context(tc.tile_pool(name="ppb", bufs=1, space="PSUM"))
        # per-head pools (stage pools stay open until interleaved stage-2 done)
        epool = ctx.enter_context(tc.tile_pool(name="eS", bufs=2))
        candp = ctx.enter_context(tc.tile_pool(name="cand", bufs=2))
        v8p = ctx.enter_context(tc.tile_pool(name="v8", bufs=2))
        v8fp = ctx.enter_context(tc.tile_pool(name="v8f", bufs=18))
        thp = ctx.enter_context(tc.tile_pool(name="theta", bufs=1))
        ethp = ctx.enter_context(tc.tile_pool(name="eth", bufs=2))
        ecp = ctx.enter_context(tc.tile_pool(name="ec", bufs=2))
        esp_p = ctx.enter_context(tc.tile_pool(name="esp", bufs=2))
        dpool = ctx.enter_context(tc.tile_pool(name="comb", bufs=1))
        zpool = ctx.enter_context(tc.tile_pool(name="z", bufs=1))
        ohpool = ctx.enter_context(tc.tile_pool(name="oh", bufs=1))
        opool = ctx.enter_context(tc.tile_pool(name="out", bufs=1))
        sctx = ExitStack()
        wqp = sctx.enter_context(tc.tile_pool(name="wqkv", bufs=1))
        xload = sctx.enter_context(tc.tile_pool(name="xload", bufs=3))
        xtp = sctx.enter_context(tc.tile_pool(name="xT", bufs=1))
        wst = sctx.enter_context(tc.tile_pool(name="wst", bufs=1))

        # ---- constants ----
        ident = consts.tile([128, 128], f32)
        make_identity(nc, ident)
        ones16 = consts.tile([1, 128], f16)
        nc.vector.memset(ones16, 1.0)

        alpha_sb = consts.tile([1, H], f32)
        nc.sync.dma_start(out=alpha_sb, in_=alpha_ext[:])
        g_sb = consts.tile([1, H], f32)
        nc.scalar.activation(g_sb, alpha_sb, AF.Sigmoid)
        gm1_sb = consts.tile([1, H], f32)  # 1 - g
        nc.vector.tensor_scalar(gm1_sb, g_sb, -1.0, 1.0, OP.mult, op1=OP.add)
        ones65 = consts.tile([65, 64], f16)  # row 64 used as K=1 bcast lhsT
        nc.vector.memset(ones65, 1.0)
        onesf = consts.tile([1, 64], f32)
        nc.vector.memset(onesf, 1.0)
        # gcol[:, h] = g_h on all 64 partitions (ACT scale operand); [:, H+h] = 1-g_h
        ps_g = ppb.tile([128, 512], f32, tag="bc")
        nc.tensor.matmul(ps_g[0:64, 0:H], onesf, g_sb)
        nc.tensor.matmul(ps_g[0:64, H:2 * H], onesf, gm1_sb)
        gcol = consts.tile([64, 2 * H], f32)
        nc.scalar.activation(gcol, ps_g[0:64, 0:2 * H], AF.Copy)

        # ---- weights ----
        wqkv_sb = []
        for kc in range(4):
            st_ = wst.tile([128, 3 * DIM], f32, tag="wqst", name="wqst")
            nc.sync.dma_start(out=st_, in_=wqkv_ext[kc * 128:(kc + 1) * 128, :])
            t_ = wqp.tile([128, 3 * DIM], f32r, tag=f"wqkv{kc}", name=f"wqkv{kc}")
            nc.scalar.activation(t_, st_, AF.Copy)
            wqkv_sb.append(t_)
        woutP = []

        def emit_wout():
            for hh in range(H):
                st_ = wst.tile([64, DIM], f32, tag="wstage")
                nc.sync.dma_start(out=st_, in_=wout_ext[hh * 64:(hh + 1) * 64, :])
                t_ = wpool.tile([64, DIM], f16, tag=f"wout{hh}", name=f"wout{hh}")
                nc.scalar.activation(t_, st_, AF.Copy)
                woutP.append(t_)

        # ---- stage 1: x -> xT [512, 2048] fp32 ----
        xTbig = xtp.tile([128, 4, T], f32r, tag="xT", name="xT")
        def emit_stage1(i):
            xt = xload.tile([128, DIM], f32, tag="xt")
            nc.sync.dma_start(out=xt, in_=x_ext[i * 128:(i + 1) * 128, :])
            ps = ppa.tile([128, 4, 128], f32, tag="mm", name="mm")
            for j in range(4):
                nc.tensor.transpose(ps[:, j, :], xt[:, j * 128:(j + 1) * 128], ident)
            nc.scalar.activation(xTbig[:, :, i * 128:(i + 1) * 128], ps, AF.Copy)

        # ---- stage 2 (fp32 matmuls): qT (scaled 1/8), kT, v -> fp16 ----
        # emitted per-m so pair-0 phase A can interleave with m=1..3 and v
        q16 = [qkp.tile([128, TQ], f16, tag=f"q{m}", name=f"q{m}") for m in range(4)]
        k16 = [qkp.tile([128, T], f16, tag=f"k{m}", name=f"k{m}") for m in range(4)]

        def emit_qk_chunk(m, j):
            # j 0..1: q chunks; j 2..5: k chunks
            if j < 2:
                nb = j
                ps = ppa.tile([128, 512], f32, tag="mm")
                for kc in range(4):
                    nc.tensor.matmul(ps, wqkv_sb[kc][:, m * 128:(m + 1) * 128],
                                     xTbig[:, kc, nb * 512:(nb + 1) * 512],
                                     start=(kc == 0), stop=(kc == 3))
                nc.scalar.activation(q16[m][:, nb * 512:(nb + 1) * 512], ps,
                                     AF.Copy, scale=0.125)
            else:
                nb = j - 2
                ps = ppa.tile([128, 512], f32, tag="mm")
                for kc in range(4):
                    nc.tensor.matmul(ps, wqkv_sb[kc][:, DIM + m * 128:DIM + (m + 1) * 128],
                                     xTbig[:, kc, nb * 512:(nb + 1) * 512],
                                     start=(kc == 0), stop=(kc == 3))
                nc.scalar.activation(k16[m][:, nb * 512:(nb + 1) * 512], ps, AF.Copy)

        def emit_qk(m):
            for j in range(6):
                emit_qk_chunk(m, j)

        v65 = []
        for hh in range(H):
            t_ = vzp.tile([128, 16, 65], f16, tag=f"v65{hh}", name=f"v65{hh}")
            nc.vector.memset(t_[:, :, 64:65], 1.0)
            v65.append(t_)

        def emit_v(st):
            ps = ppa.tile([128, 512], f32, tag="mm")
            for kc in range(4):
                nc.tensor.matmul(ps, xTbig[:, kc, st * 128:(st + 1) * 128],
                                 wqkv_sb[kc][:, 2 * DIM:3 * DIM],
                                 start=(kc == 0), stop=(kc == 3))
            for hh in range(H):
                nc.scalar.activation(v65[hh][:, st, 0:64],
                                     ps[:, hh * 64:(hh + 1) * 64], AF.Copy)

        oh16 = [ohpool.tile([64, TQ], f16, tag=f"oh{hh}", name=f"oh{hh}")
                for hh in range(H)]

        # ---- software-pipelined main loop ----
        # emit_A(p, tt): S matmuls + exp + tournament for both heads of a
        #   128-query-row tile. Round 1 = top-8 per 64-wide segment (32 max8,
        #   no replace): candidates hold the exact top-64 unless a segment
        #   contains >8 of them (~5e-3 of rows, over-include only).
        # emit_B(p): tau rows -> ETh broadcast.
        # emit_C(p, tb, sc): S^T chunk -> e -> dense PV; mask -> sparse PV.
        # emit_D(p, tb): combine into oh16.
        # Schedule: A(0); B(0); then for each p: interleave A(p+1) tiles with
        # C(p) chunks so the DVE tournament of p+1 overlaps PE/ACT of C(p).
        thc = {}
        ETh = {}

        eS_live = {}

        def emit_A_mm(p, tt):
            pair = (2 * p, 2 * p + 1)
            eS = {}
            for hh in pair:
                eS[hh] = epool.tile([128, T], f16, tag=f"eS{hh % 2}",
                                    name=f"eS{hh % 2}")
            for nb in range(4):
                for hh in pair:
                    rr = (hh % 2) * 64
                    ps = ppa.tile([128, 512], f32, tag="mm")
                    nc.tensor.matmul(ps, q16[p][rr:rr + 64, tt * 128:(tt + 1) * 128],
                                     k16[p][rr:rr + 64, nb * 512:(nb + 1) * 512])
                    nc.scalar.activation(eS[hh][:, nb * 512:(nb + 1) * 512],
                                         ps, AF.Exp)
            eS_live[p] = eS

        def emit_A_tour(p, tt):
            pair = (2 * p, 2 * p + 1)
            eS = eS_live[p]
            for hh in pair:
                if tt == 7 and hh == pair[1]:
                    emit_B_head(pair[0])
                e_t = eS[hh]
                cand = candp.tile([128, 176], f16, tag=f"cand{hh % 2}",
                                  name=f"cand{hh % 2}")
                for gs, (lo, hi) in enumerate(SEGB):
                    nc.vector.max(out=cand[:, gs * 8:gs * 8 + 8],
                                  in_=e_t[:, lo:hi])
                for r in range(7):
                    v8 = v8p.tile([128, 8], f16, tag=f"v8{hh % 2}",
                                  name=f"v8{hh % 2}")
                    nc.vector.max(out=v8, in_=cand)
                    nc.vector.match_replace(out=cand, in_to_replace=v8,
                                            in_values=cand, imm_value=0.0)
                v8f = v8fp.tile([128, 8], f32, tag="v8f", name="v8f")
                nc.vector.max(out=v8f, in_=cand)
                thc[hh].append(v8f[:, 7:8])
            if tt == 7:
                emit_B_head(pair[1])

        def emit_B_head(hh):
            thetaR = thp.tile([1, TQ], f16, tag=f"thR{hh % 2}",
                              name=f"thR{hh % 2}")
            for half in range(2):
                psr = ppb.tile([128, 512], f32, tag="bc")
                for q4 in range(4):
                    tt = half * 4 + q4
                    nc.tensor.transpose(psr[0:1, q4 * 128:(q4 + 1) * 128],
                                        thc[hh][tt], ident)
                nc.scalar.activation(thetaR[0:1, half * 512:(half + 1) * 512],
                                     psr[0:1, :], AF.Copy)
            ETh[hh % 2] = ethp.tile([128, TQ], f16, tag=f"ETh{hh % 2}",
                                    name=f"ETh{hh % 2}")
            for nb in range(2):
                psb = ppb.tile([128, 512], f32, tag="bc")
                nc.tensor.matmul(psb, ones16,
                                 thetaR[0:1, nb * 512:(nb + 1) * 512])
                nc.scalar.activation(ETh[hh % 2][:, nb * 512:(nb + 1) * 512],
                                     psb, AF.Copy)

        def emit_C(p, tb, sc, den, sp):
            pair = (2 * p, 2 * p + 1)
            tbs = slice(tb * 512, (tb + 1) * 512)
            pss, ec = {}, {}
            for hh in pair:
                rr = (hh % 2) * 64
                ps = ppa.tile([128, 512], f32, tag="mm")
                nc.tensor.matmul(ps, k16[p][rr:rr + 64, sc * 128:(sc + 1) * 128],
                                 q16[p][rr:rr + 64, tbs])
                pss[hh] = ps
            for hh in pair:
                ec[hh] = ecp.tile([128, 512], f16, tag=f"ec{hh % 2}",
                                  name=f"ec{hh % 2}")
                nc.scalar.activation(ec[hh], pss[hh], AF.Exp)
                nc.tensor.matmul(den[hh][0:65, :], v65[hh][:, sc, :], ec[hh],
                                 start=(sc == 0), stop=(sc == 15))
            for hh in pair:
                es = esp_p.tile([128, 512], f16, tag=f"esp{hh % 2}",
                                name=f"esp{hh % 2}")
                nc.vector.tensor_tensor(es, ec[hh], ETh[hh % 2][:, tbs],
                                        op=OP.is_ge)
                nc.vector.tensor_tensor(es, es, ec[hh], op=OP.mult)
                nc.tensor.matmul(sp[hh][0:65, :], v65[hh][:, sc, :], es,
                                 start=(sc == 0), stop=(sc == 15))

        def emit_D(p, tb, den, sp):
            pair = (2 * p, 2 * p + 1)
            tbs = slice(tb * 512, (tb + 1) * 512)
            for hh in pair:
                d16 = dpool.tile([64, 512], f16, tag=f"d16{hh % 2}",
                                 name=f"d16{hh % 2}")
                nc.scalar.activation(d16, den[hh][0:64, :], AF.Copy,
                                     scale=gcol[:, hh:hh + 1])
                s16 = dpool.tile([64, 512], f16, tag=f"s16{hh % 2}",
                                 name=f"s16{hh % 2}")
                nc.scalar.activation(s16, sp[hh][0:64, :], AF.Copy,
                                     scale=gcol[:, H + hh:H + hh + 1])
                zr = zpool.tile([65, 2 * 512], f16, tag=f"zr{hh % 2}",
                                name=f"zr{hh % 2}")
                nc.scalar.activation(zr[64:65, 0:512], den[hh][64:65, :], AF.Copy)
                nc.scalar.activation(zr[64:65, 512:1024], sp[hh][64:65, :], AF.Copy)
                bcd_ps = ppb.tile([128, 512], f32, tag="bc")
                nc.tensor.matmul(bcd_ps[0:64, :], ones65[64:65, :],
                                 zr[64:65, 0:512])
                zbd = zpool.tile([64, 512], f32, tag=f"zbd{hh % 2}",
                                 name=f"zbd{hh % 2}")
                nc.vector.reciprocal_approx_fast(zbd, bcd_ps[0:64, :])
                bcd = dpool.tile([64, 512], f16, tag=f"bcd{hh % 2}",
                                 name=f"bcd{hh % 2}")
                nc.scalar.activation(bcd, zbd, AF.Copy)
                bcs_ps = ppb.tile([128, 512], f32, tag="bc")
                nc.tensor.matmul(bcs_ps[0:64, :], ones65[64:65, :],
                                 zr[64:65, 512:1024])
                zbs = zpool.tile([64, 512], f32, tag=f"zbs{hh % 2}",
                                 name=f"zbs{hh % 2}")
                nc.vector.reciprocal_approx_fast(zbs, bcs_ps[0:64, :])
                bcs = dpool.tile([64, 512], f16, tag=f"bcs{hh % 2}",
                                 name=f"bcs{hh % 2}")
                nc.scalar.activation(bcs, zbs, AF.Copy)
                tmp = dpool.tile([64, 512], f16, tag=f"tmp{hh % 2}",
                                 name=f"tmp{hh % 2}")
                nc.vector.tensor_tensor(tmp, d16, bcd, op=OP.mult)
                nc.vector.tensor_tensor(oh16[hh][:, tbs], s16, bcs, op=OP.mult)
                nc.vector.tensor_add(oh16[hh][:, tbs], oh16[hh][:, tbs], tmp)

        def emit_out(tt):
            psA = ppa.tile([128, 512], f32, tag="mm")
            for hh in range(H):
                nc.tensor.matmul(psA, oh16[hh][:, tt * 128:(tt + 1) * 128],
                                 woutP[hh], start=(hh == 0), stop=(hh == 7))
            o_sb = opool.tile([128, DIM], f32, tag="osb")
            nc.scalar.activation(o_sb, psA, AF.Copy)
            nc.sync.dma_start(out=out_ext[tt * 128:(tt + 1) * 128, :], in_=o_sb)

        thc = {0: [], 1: []}
        for blk in range(4):
            for i4 in range(4):
                emit_stage1(blk * 4 + i4)
            # q0/k0 chunks over t-range blk*512 are ready now
            if blk in (0, 2):
                emit_qk_chunk(0, blk // 2)
            emit_qk_chunk(0, 2 + blk)
        stage2_rest = ([("qk", (m, j)) for m in (1, 2, 3) for j in range(6)]
                       + [("v", st) for st in range(16)])
        si = 0
        takes = [5, 5, 4, 4, 4, 4, 4, 4]
        for tt in range(8):
            emit_A_mm(0, tt)
            for _ in range(takes[tt]):
                if si < len(stage2_rest):
                    kind, arg = stage2_rest[si]
                    si += 1
                    if kind == "qk":
                        emit_qk_chunk(*arg)
                    else:
                        emit_v(arg)
            emit_A_tour(0, tt)
        assert si == len(stage2_rest)
        emit_wout()
        sctx.close()
        for p in range(4):
            den, sp = {}, {}
            nxt = p + 1
            if nxt < 4:
                thc = {2 * nxt: [], 2 * nxt + 1: []}
            for tt in range(8):
                ci0 = tt * 4
                for ci in range(ci0, ci0 + 4):
                    tb, sc = ci // 16, ci % 16
                    if sc == 0:
                        for hh in (2 * p, 2 * p + 1):
                            den[hh] = ppv.tile([65, 512], f32, tag=f"den{hh % 2}",
                                               name=f"den{hh % 2}")
                            sp[hh] = ppv.tile([65, 512], f32, tag=f"sp{hh % 2}",
                                              name=f"sp{hh % 2}")
                    emit_C(p, tb, sc, den, sp)
                    if sc == 15:
                        emit_D(p, tb, den, sp)
                if nxt < 4:
                    emit_A_mm(nxt, tt)
                if p == 3 and tt >= 4:
                    emit_out(tt - 4)
                if nxt < 4:
                    emit_A_tour(nxt, tt)
        for tt in range(4, 8):
            emit_out(tt)


    nc.finalize()
    return nc


def kernel(x, Wqkv, Wout, alpha):
    global LAST_EXEC_NS
    from concourse.bass_utils import run_bass_kernel_spmd

    if "nc" not in _CACHE:
        _CACHE["nc"] = _build_nc()
    nc = _CACHE["nc"]

    x = np.ascontiguousarray(np.asarray(x, dtype=np.float32))
    wqkv = np.ascontiguousarray(np.asarray(Wqkv, dtype=np.float32))
    wout = np.ascontiguousarray(np.asarray(Wout, dtype=np.float32))
    al = np.ascontiguousarray(np.asarray(alpha, dtype=np.float32).reshape(1, H))

    in_maps = []
    for c in range(NCORES):
        b, th = c // 2, c % 2
        xb = x[b]
        if th == 1:
            xb = np.ascontiguousarray(np.concatenate([xb[TQ:], xb[:TQ]], axis=0))
        in_maps.append({"x": xb, "wqkv": wqkv, "wout": wout, "alpha": al})

    trace = bool(int(os.environ.get("KERNEL_PROFILE", "0")))
    if trace:
        # this container's antenv lacks axon_hooks; shim it with the ctypes
        # NTFF hook from trn_agent_boot so neuron-profile tracing works
        import sys as _sys, types as _types
        if "antenv.axon_hooks" not in _sys.modules:
            try:
                from antenv.axon_hooks import get_axon_ntff_profile_hook  # noqa
            except ImportError:
                _hook = None
                try:
                    from trn_agent_boot.trn_boot import _ntff_profile_via_ctypes
                    _hook = _ntff_profile_via_ctypes("/opt/axon/libaxon_pjrt.so")
                except Exception:
                    _hook = None
                _m = _types.ModuleType("antenv.axon_hooks")
                _m.get_axon_ntff_profile_hook = lambda: _hook
                _sys.modules["antenv.axon_hooks"] = _m
    res = run_bass_kernel_spmd(nc, in_maps, core_ids=list(range(NCORES)),
                               trace=trace)
    LAST_EXEC_NS = res.exec_time_ns
    if trace:
        _CACHE["last_results"] = res

    out = np.empty((B, T, DIM), np.float32)
    for c in range(NCORES):
        b, th = c // 2, c % 2
        out[b, th * TQ:(th + 1) * TQ, :] = res.results[c]["out"]
    return out
